# revision 33
# baseline (speedup 1.0000x reference)
"""Causal self-attention (QKV proj + RoPE + causal softmax attention +
output proj + LayerNorm) on 8 Trainium2 NeuronCores.

Sharding: core c handles batch b = c//4 and head group g = c%4 (4 of the
16 heads). Each core projects QKV for its heads from the full sequence,
runs flash-style attention in transposed ([dim, token]) layout, computes
its partial c_proj contribution, ReduceScatters partials over the 4 cores
of its batch in 512-token chunks (overlapped with later attention),
LayerNorms each received 128-token chunk, and returns [512, 1024] rows.
Host assembles the [2, 2048, 1024] output.

Precision: projections in float32r (full-precision storage, tf32-like
matmul); attention scores/probs in bf16; softmax/normalizations in f32.
"""
import sys

sys.path.insert(0, "/opt/trn_rl_repo")

import numpy as np

import concourse.bass as bass
import concourse.tile as tile
from concourse.tile_rust import add_dep_helper
from concourse import bacc, mybir
from concourse.bass_utils import run_bass_kernel_spmd

B, T, C, H = 2, 2048, 1024, 16
D = C // H              # 64
HPC = 4                 # heads per core
DC = HPC * D            # 256 local q/k/v dims per core
N_CORES = 8
EPS = 1e-5

F32 = mybir.dt.float32
F32R = mybir.dt.float32r
BF16 = mybir.dt.bfloat16

TT = 512                # token tile (moving dim) for qkv/attention
NTT = T // TT           # 4
NTB = T // 128          # 16 token blocks


def build():
    nc = bacc.Bacc("TRN2", target_bir_lowering=False, debug=False,
                   num_devices=N_CORES)

    xT_d = nc.dram_tensor("xT", [C, T], BF16, kind="ExternalInput").ap()
    wqkvT_d = nc.dram_tensor("wqkvT", [C, 3 * DC], BF16, kind="ExternalInput").ap()
    wpT_d = nc.dram_tensor("wpT", [C, C], F32R, kind="ExternalInput").ap()
    c_d = nc.dram_tensor("c128", [128, T], F32, kind="ExternalInput").ap()
    s_d = nc.dram_tensor("s128", [128, T], F32, kind="ExternalInput").ap()
    r_d = nc.dram_tensor("rmat", [128, 128], F32R, kind="ExternalInput").ap()
    mask_d = nc.dram_tensor("mask", [128, 128], BF16, kind="ExternalInput").ap()
    lnw_d = nc.dram_tensor("ln_w", [C], F32, kind="ExternalInput").ap()
    lnb_d = nc.dram_tensor("ln_b", [C], F32, kind="ExternalInput").ap()
    ones_d = nc.dram_tensor("ones", [1, 64], F32R, kind="ExternalInput").ap()
    boff_d = nc.dram_tensor("blk_off", [1, 1], mybir.dt.uint32,
                            kind="ExternalInput").ap()
    out_d = nc.dram_tensor("out", [T // 4, C], F32, kind="ExternalOutput").ap()

    with tile.TileContext(nc) as tc:
        with (
            tc.tile_pool(name="consts", bufs=1) as consts,
            tc.tile_pool(name="persist", bufs=1) as persist,
            tc.tile_pool(name="work", bufs=1) as work,
            tc.tile_pool(name="ps", bufs=1, space="PSUM") as ps,
            tc.tile_pool(name="dram", bufs=1, space="DRAM") as dram,
        ):
            # ---- constants -------------------------------------------------
            wqkv_sb = consts.tile([128, 8, 3 * DC], BF16)
            for kc in range(8):
                nc.sync.dma_start(
                    out=wqkv_sb[:, kc, :],
                    in_=wqkvT_d.rearrange("(kc p) n -> p kc n", p=128)[:, kc, :])
            xt0 = work.tile([128, 8, TT], BF16, name="xt", bufs=2)
            for kc in range(8):
                nc.sync.dma_start(
                    out=xt0[:, kc, :],
                    in_=xT_d.rearrange("(kc p) t -> p kc t", p=128)[:, kc, 0:TT],
                )
            c_sb = consts.tile([128, T], F32)
            nc.sync.dma_start(out=c_sb[:], in_=c_d[:])
            s_sb = consts.tile([128, T], F32)
            nc.sync.dma_start(out=s_sb[:], in_=s_d[:])
            r_sb = consts.tile([128, 128], F32R)
            nc.sync.dma_start(out=r_sb[:], in_=r_d[:])
            mask_sb = consts.tile([128, 128], BF16)
            nc.sync.dma_start(out=mask_sb[:], in_=mask_d[:])
            lnw_sb = consts.tile([128, C], F32)
            nc.sync.dma_start(out=lnw_sb[:], in_=bass.AP(
                tensor=lnw_d.tensor, offset=lnw_d.offset,
                ap=[[0, 128]] + list(lnw_d.ap)))
            lnb_sb = consts.tile([128, C], F32)
            nc.sync.dma_start(out=lnb_sb[:], in_=bass.AP(
                tensor=lnb_d.tensor, offset=lnb_d.offset,
                ap=[[0, 128]] + list(lnb_d.ap)))
            ones_sb = consts.tile([1, 64], F32R)
            nc.sync.dma_start(out=ones_sb[:], in_=ones_d[:])
            eps_sb = consts.tile([128, 1], F32)
            nc.vector.memset(eps_sb[:], EPS)
            boff_sb = consts.tile([1, 1], mybir.dt.uint32)
            nc.sync.dma_start(out=boff_sb[:], in_=boff_d[:])
            warm_in = dram.tile([8, 32], F32, name="warm_in")
            warm_out = dram.tile([8, 32], F32, name="warm_out")
            warm_sb = consts.tile([8, 32], F32, name="warm_sb")
            nc.vector.memset(warm_sb[:], 0.0)
            nc.gpsimd.dma_start(out=warm_in[:], in_=warm_sb[:])
            nc.gpsimd.collective_compute(
                "AllToAll", mybir.AluOpType.bypass,
                replica_groups=[[0, 1, 2, 3, 4, 5, 6, 7]],
                ins=[warm_in.opt()], outs=[warm_out.opt()],
            )

            # ---- persistent activations -----------------------------------
            qrot = persist.tile([128, 2, T], BF16)   # [:, i, :] = heads 2i,2i+1
            krot = persist.tile([128, 2, T], BF16)
            v_sb = persist.tile([128, NTB, HPC, D + 1], BF16)
            nc.vector.memset(v_sb[:, :, :, D:D + 1], 1.0)

            # ---- phase 1 (interleaved): QKV projection + RoPE, and V ------
            # col blocks 0..1: q heads (01),(23); 2..3: k; cols 512:768: v
            def qkv_tile(tt):
                if tt == 0:
                    xt = xt0
                else:
                    xt = work.tile([128, 8, TT], BF16, name="xt", bufs=2)
                    for kc in range(8):
                        nc.sync.dma_start(
                            out=xt[:, kc, :],
                            in_=xT_d.rearrange("(kc p) t -> p kc t", p=128)[
                                :, kc, tt * TT:(tt + 1) * TT],
                        )
                for cb in range(4):
                    qk_ps = ps.tile([128, TT], F32, name="qk_ps", tag="big1024",
                                    bufs=2, padded_shape=[128, 1024])
                    for kc in range(8):
                        nc.tensor.matmul(
                            qk_ps[:],
                            wqkv_sb[:, kc, cb * 128:(cb + 1) * 128],
                            xt[:, kc, :],
                            start=(kc == 0),
                            stop=(kc == 7),
                        )
                    qkT = work.tile([128, TT], F32R, name="qkT", bufs=2)
                    nc.scalar.copy(qkT[:], qk_ps[:])
                    u_ps = ps.tile([128, TT], F32, name="u_ps", tag="reptag",
                                   bufs=2)
                    nc.tensor.matmul(u_ps[:], r_sb[:], qkT[:],
                                     start=True, stop=True)
                    dest = (qrot if cb < 2 else krot)[:, cb % 2,
                                                      tt * TT:(tt + 1) * TT]
                    csl = c_sb[:, tt * TT:(tt + 1) * TT]
                    ssl = s_sb[:, tt * TT:(tt + 1) * TT]
                    tmp = work.tile([128, TT], F32, name="tmp", bufs=2)
                    tmp2 = work.tile([128, TT], F32, name="tmp2", bufs=2)
                    nc.vector.tensor_mul(tmp[:], qkT[:], csl)
                    nc.vector.tensor_mul(tmp2[:], u_ps[:], ssl)
                    nc.vector.tensor_add(dest, tmp[:], tmp2[:])
                # V for the 4 token blocks of this token tile
                for tb in range(tt * 4, tt * 4 + 4):
                    v_ps = ps.tile([128, DC], F32, name="v_ps", tag="big1024",
                                   bufs=2, padded_shape=[128, 1024])
                    for kc in range(8):
                        nc.tensor.matmul(
                            v_ps[:],
                            xt[:, kc, (tb % 4) * 128:(tb % 4 + 1) * 128],
                            wqkv_sb[:, kc, 2 * DC:3 * DC],
                            start=(kc == 0),
                            stop=(kc == 7),
                        )
                    nc.scalar.copy(
                        v_sb[:, tb, :, 0:D],
                        v_ps[:].rearrange("p (h d) -> p h d", h=HPC),
                    )

            qkv_tile(0)

            # weights for c_proj (first needed ~halfway through attention)
            wp_sb = consts.tile([128, 8, C], F32R)
            nc.sync.dma_start(out=wp_sb[:],
                              in_=wpT_d.rearrange("(kc p) n -> p kc n", p=128))

            # ---- phase 2: per 512-token chunk: attention (4 heads) +
            #      c_proj partial + chunked ReduceScatter + LayerNorm -------
            scale = 1.0 / float(np.sqrt(D))
            boff_reg = nc.gpsimd.alloc_register("blkoff")
            nc.gpsimd.reg_load(boff_reg, boff_sb[0:1, 0:1])
            boff_val = nc.gpsimd.snap(boff_reg, donate=True, min_val=0,
                                      max_val=4)
            pend = None

            def cproj_ln(a2a_outs, qt, att_mm, tail=False):
                yTg = work.tile([128, 8, 128], F32R, name="yTg", bufs=2)
                for h2 in range(2):
                    ld = nc.gpsimd.dma_start(
                        out=yTg[:].rearrange("p (r hh) t -> p r hh t",
                                             hh=2)[:, :, h2, :].bitcast(F32),
                        in_=a2a_outs[h2][bass.ds(boff_val, 4)].rearrange(
                            "r p t -> p r t"),
                    )
                    add_dep_helper(ld.ins, att_mm.ins, sync=False,
                                   reason="pipeline c_proj one chunk behind")
                ln = work.tile([128, C], F32, name="ln", bufs=2)
                for oc in range(2):
                    co_ps = ps.tile([128, 512], F32, name="co_ps",
                                    tag="big1024", bufs=2,
                                    padded_shape=[128, 1024])
                    for i, kc in enumerate((0, 2, 4, 6, 1, 3, 5, 7)):
                        nc.tensor.matmul(
                            co_ps[:],
                            yTg[:, kc, :],
                            wp_sb[:, kc, oc * 512:(oc + 1) * 512],
                            start=(i == 0),
                            stop=(i == 7),
                            skip_group_check=True,
                        )
                    nc.vector.tensor_copy(ln[:, oc * 512:(oc + 1) * 512],
                                          co_ps[:])
                stats = work.tile([128, 2, 6], F32, name="stats", bufs=2)
                nc.vector.bn_stats(stats[:, 0, :], ln[:, 0:512])
                nc.vector.bn_stats(stats[:, 1, :], ln[:, 512:1024])
                mv = work.tile([128, 2], F32, name="mv", bufs=2)
                nc.vector.bn_aggr(mv[:], stats[:])
                negmean = work.tile([128, 1], F32, name="negmean", bufs=2)
                nc.vector.tensor_scalar_mul(negmean[:], mv[:, 0:1], -1.0)
                std = work.tile([128, 1], F32, name="std", bufs=2)
                nc.scalar.activation(std[:], mv[:, 1:2],
                                     mybir.ActivationFunctionType.Sqrt,
                                     bias=eps_sb[:])
                rstd = work.tile([128, 1], F32, name="rstd", bufs=2)
                nc.vector.reciprocal_approx_fast(rstd[:], std[:])
                nc.vector.tensor_scalar(
                    ln[:], ln[:], negmean[:], rstd[:],
                    op0=mybir.AluOpType.add, op1=mybir.AluOpType.mult,
                )
                eng = nc.vector if tail else nc.gpsimd
                eng.tensor_mul(ln[:], ln[:], lnw_sb[:])
                eng.tensor_add(ln[:], ln[:], lnb_sb[:])
                nc.gpsimd.dma_start(out=out_d[qt * 128:(qt + 1) * 128, :],
                                    in_=ln[:])
            for qt in (0, 1, 2, 3):
                y_qt = work.tile([128, 2, TT], F32R, name="y_qt", bufs=2)
                a2a_outs = []
                for hp in range(2):
                    hA, hB = 2 * hp, 2 * hp + 1
                    qsrc = qrot[:, hp, :]
                    ksrc = krot[:, hp, :]
                    y_psA = ps.tile([D + 1, TT], F32, name="y_psA",
                                    tag="ytag", bufs=2)
                    y_psB = ps.tile([D + 1, TT], F32, name="y_psB",
                                    tag="ytag", bufs=2)
                    for kb in range(4 * qt + 4):
                        s_off = kb - 4 * qt      # >= 0 -> diagonal block
                        q_lo = 128 * s_off if s_off > 0 else 0
                        sc_ps = ps.tile([128, 2, TT], F32, name="sc_ps",
                                        tag="big1024", bufs=2)
                        nc.tensor.matmul(
                            sc_ps[:, 0, q_lo:],
                            ksrc[0:64, kb * 128:(kb + 1) * 128],
                            qsrc[0:64, qt * TT + q_lo:(qt + 1) * TT],
                            start=True, stop=True,
                        )
                        nc.tensor.matmul(
                            sc_ps[:, 1, q_lo:],
                            ksrc[64:128, kb * 128:(kb + 1) * 128],
                            qsrc[64:128, qt * TT + q_lo:(qt + 1) * TT],
                            start=True, stop=True,
                        )
                        ex = work.tile([128, 2, TT], BF16, name="ex", bufs=3)
                        last_exp = nc.scalar.activation(
                            ex[:, :, q_lo:], sc_ps[:, :, q_lo:],
                            mybir.ActivationFunctionType.Exp, scale=scale,
                        )
                        if s_off >= 0:
                            nc.vector.tensor_mul(
                                ex[:, 0, q_lo:q_lo + 128],
                                ex[:, 0, q_lo:q_lo + 128],
                                mask_sb[:],
                            )
                            nc.vector.tensor_mul(
                                ex[:, 1, q_lo:q_lo + 128],
                                ex[:, 1, q_lo:q_lo + 128],
                                mask_sb[:],
                            )
                        nc.tensor.matmul(
                            y_psA[:, q_lo:],
                            v_sb[:, kb, hA, :],
                            ex[:, 0, q_lo:],
                            start=(kb == 0),
                            stop=(kb == 4 * qt + 3),
                            skip_group_check=True,
                        )
                        last_att_mm = nc.tensor.matmul(
                            y_psB[:, q_lo:],
                            v_sb[:, kb, hB, :],
                            ex[:, 1, q_lo:],
                            start=(kb == 0),
                            stop=(kb == 4 * qt + 3),
                            skip_group_check=True,
                        )
                    # normalize each head: y / denom (denom = row D of y_ps)
                    for half, y_ps in ((0, y_psA), (1, y_psB)):
                        den = work.tile([1, TT], F32R, name="den", bufs=2)
                        nc.vector.tensor_copy(den[:], y_ps[D:D + 1, :])
                        rep_ps = ps.tile([64, TT], F32, name="rep_ps",
                                         tag="reptag", bufs=2)
                        nc.tensor.matmul(rep_ps[:], ones_sb[:], den[:],
                                         start=True, stop=True)
                        rrec = work.tile([64, TT], F32, name="rrec", bufs=2)
                        nc.vector.reciprocal_approx_fast(rrec[:], rep_ps[:])
                        nc.vector.tensor_mul(
                            y_qt[64 * half:64 * half + 64, hp, :],
                            y_ps[0:D, :],
                            rrec[:],
                        )
                    # ship this head-pair's y through AllToAll: every core
                    # gets these 128 y-dims for its own 128-token slice
                    a2a_in = dram.tile([8, 128, 128], F32,
                                       name=f"a2a_in{qt}_{hp}")
                    for half in range(2):
                        nc.gpsimd.dma_start(
                            out=a2a_in[4 * half:4 * half + 4].rearrange(
                                "r p t -> p r t"),
                            in_=y_qt[:, hp, :].rearrange(
                                "p (r t) -> p r t", r=4).bitcast(F32),
                        )
                    a2a_out = dram.tile([8, 128, 128], F32,
                                        name=f"a2a_out{qt}_{hp}")
                    a2a_outs.append(a2a_out)
                    nc.gpsimd.collective_compute(
                        "AllToAll",
                        mybir.AluOpType.bypass,
                        replica_groups=[[0, 1, 2, 3, 4, 5, 6, 7]],
                        ins=[a2a_in.opt()],
                        outs=[a2a_out.opt()],
                    )

                if qt < 3:
                    qkv_tile(qt + 1)
                if pend is not None:
                    cproj_ln(*pend, last_att_mm)
                pend = (a2a_outs, qt)
            cproj_ln(*pend, last_att_mm, tail=True)

    nc.compile()
    return nc


_PERM64 = np.concatenate([np.arange(0, 64, 2), np.arange(1, 64, 2)])


def _host_prep(x, rope_freqs, W_attn, W_proj, ln_weight, ln_bias):
    """Build the 8 per-core input maps."""
    import ml_dtypes

    x = np.ascontiguousarray(np.asarray(x, dtype=np.float32))
    W_attn = np.asarray(W_attn, dtype=np.float32)
    W_proj = np.asarray(W_proj, dtype=np.float32)
    rope_freqs = np.asarray(rope_freqs, dtype=np.float32)
    ln_weight = np.ascontiguousarray(np.asarray(ln_weight, dtype=np.float32))
    ln_bias = np.ascontiguousarray(np.asarray(ln_bias, dtype=np.float32))

    # RoPE tables, [128, T]: row r uses pair frequency freqs[r % 32]
    t = np.arange(T, dtype=np.float32)
    theta = t[None, :] * rope_freqs[np.arange(128) % 32][:, None]
    c128 = np.cos(theta).astype(np.float32)
    s128 = np.sin(theta).astype(np.float32)

    # rotation helper matrix (lhsT layout [K, M]), per 64-row head block
    rmat = np.zeros((128, 128), dtype=np.float32)
    for j2 in range(2):
        base = 64 * j2
        for r in range(32):
            rmat[base + 32 + r, base + r] = -1.0
            rmat[base + r, base + 32 + r] = 1.0

    mask = np.triu(np.ones((128, 128))).astype(ml_dtypes.bfloat16)

    xT = [np.ascontiguousarray(x[b].T).astype(ml_dtypes.bfloat16) for b in range(B)]
    wpT_full = np.ascontiguousarray(W_proj.T)

    in_maps = []
    for c in range(N_CORES):
        b, g = c // 4, c % 4
        heads = range(4 * g, 4 * g + 4)
        wq = np.concatenate([W_attn[h * D + _PERM64] for h in heads])
        wk = np.concatenate([W_attn[C + h * D + _PERM64] for h in heads])
        wv = W_attn[2 * C + 4 * g * D:2 * C + (4 * g + 4) * D]
        wqkvT = np.ascontiguousarray(np.concatenate([wq, wk, wv]).T).astype(ml_dtypes.bfloat16)

        in_maps.append({
            "xT": xT[b],
            "wqkvT": wqkvT,
            "wpT": wpT_full,
            "c128": c128,
            "s128": s128,
            "rmat": rmat,
            "mask": mask,
            "ln_w": ln_weight,
            "ln_b": ln_bias,
            "ones": np.ones((1, 64), dtype=np.float32),
            "blk_off": np.array([[4 * b]], dtype=np.uint32),
        })
    return in_maps


_NC_CACHE = None


def kernel(x, rope_freqs, W_attn, W_proj, ln_weight, ln_bias):
    global _NC_CACHE
    if _NC_CACHE is None:
        _NC_CACHE = build()
    nc = _NC_CACHE
    in_maps = _host_prep(x, rope_freqs, W_attn, W_proj, ln_weight, ln_bias)
    res = None
    for attempt in range(3):
        try:
            res = run_bass_kernel_spmd(nc, in_maps,
                                       core_ids=list(range(N_CORES)))
            break
        except Exception:
            if attempt == 2:
                raise
    out = np.empty((B, T, C), dtype=np.float32)
    for c in range(N_CORES):
        b, g = c // 4, c % 4
        chunk = res.results[c]["out"]     # [512, 1024]: 4 chunks of 128 rows
        for qt in range(NTT):
            out[b, qt * 512 + g * 128:qt * 512 + g * 128 + 128, :] = \
                chunk[qt * 128:(qt + 1) * 128]
    return out


# revision 34
# speedup vs baseline: 1.0786x; 1.0786x over previous
"""Causal self-attention (QKV proj + RoPE + causal softmax attention +
output proj + LayerNorm) on 8 Trainium2 NeuronCores.

Sharding: core c handles batch b = c//4 and head group g = c%4 (4 of the
16 heads). Each core projects QKV for its heads from the full sequence,
runs flash-style attention in transposed ([dim, token]) layout, computes
its partial c_proj contribution, ReduceScatters partials over the 4 cores
of its batch in 512-token chunks (overlapped with later attention),
LayerNorms each received 128-token chunk, and returns [512, 1024] rows.
Host assembles the [2, 2048, 1024] output.

Precision: projections in float32r (full-precision storage, tf32-like
matmul); attention scores/probs in bf16; softmax/normalizations in f32.
"""
import sys

sys.path.insert(0, "/opt/trn_rl_repo")

import numpy as np

import concourse.bass as bass
import concourse.tile as tile
from concourse.tile_rust import add_dep_helper
from concourse import bacc, mybir
from concourse.bass_utils import run_bass_kernel_spmd

B, T, C, H = 2, 2048, 1024, 16
D = C // H              # 64
HPC = 4                 # heads per core
DC = HPC * D            # 256 local q/k/v dims per core
N_CORES = 8
EPS = 1e-5

F32 = mybir.dt.float32
F32R = mybir.dt.float32r
BF16 = mybir.dt.bfloat16

TT = 512                # token tile (moving dim) for qkv/attention
NTT = T // TT           # 4
NTB = T // 128          # 16 token blocks


def build():
    nc = bacc.Bacc("TRN2", target_bir_lowering=False, debug=False,
                   num_devices=N_CORES)

    xT_d = nc.dram_tensor("xT", [C, T], BF16, kind="ExternalInput").ap()
    wqkvT_d = nc.dram_tensor("wqkvT", [C, 3 * DC], BF16, kind="ExternalInput").ap()
    wpT_d = nc.dram_tensor("wpT", [C, C], F32R, kind="ExternalInput").ap()
    c_d = nc.dram_tensor("c128", [128, T], F32, kind="ExternalInput").ap()
    s_d = nc.dram_tensor("s128", [128, T], F32, kind="ExternalInput").ap()
    r_d = nc.dram_tensor("rmat", [128, 128], F32R, kind="ExternalInput").ap()
    mask_d = nc.dram_tensor("mask", [128, 128], BF16, kind="ExternalInput").ap()
    lnw_d = nc.dram_tensor("ln_w", [C], F32, kind="ExternalInput").ap()
    lnb_d = nc.dram_tensor("ln_b", [C], F32, kind="ExternalInput").ap()
    ones_d = nc.dram_tensor("ones", [1, 64], F32R, kind="ExternalInput").ap()
    boff_d = nc.dram_tensor("blk_off", [1, 1], mybir.dt.uint32,
                            kind="ExternalInput").ap()
    out_d = nc.dram_tensor("out", [T // 4, C], F32, kind="ExternalOutput").ap()

    with tile.TileContext(nc) as tc:
        with (
            tc.tile_pool(name="consts", bufs=1) as consts,
            tc.tile_pool(name="persist", bufs=1) as persist,
            tc.tile_pool(name="work", bufs=1) as work,
            tc.tile_pool(name="ps", bufs=1, space="PSUM") as ps,
            tc.tile_pool(name="dram", bufs=1, space="DRAM") as dram,
        ):
            # ---- constants -------------------------------------------------
            wqkv_sb = consts.tile([128, 8, 3 * DC], BF16)
            for kc in range(8):
                nc.sync.dma_start(
                    out=wqkv_sb[:, kc, :],
                    in_=wqkvT_d.rearrange("(kc p) n -> p kc n", p=128)[:, kc, :])
            xt0 = work.tile([128, 8, TT], BF16, name="xt", bufs=2)
            for kc in range(8):
                nc.sync.dma_start(
                    out=xt0[:, kc, :],
                    in_=xT_d.rearrange("(kc p) t -> p kc t", p=128)[:, kc, 0:TT],
                )
            c_sb = consts.tile([128, T], F32)
            nc.sync.dma_start(out=c_sb[:], in_=c_d[:])
            s_sb = consts.tile([128, T], F32)
            nc.sync.dma_start(out=s_sb[:], in_=s_d[:])
            r_sb = consts.tile([128, 128], F32R)
            nc.sync.dma_start(out=r_sb[:], in_=r_d[:])
            mask_sb = consts.tile([128, 128], BF16)
            nc.sync.dma_start(out=mask_sb[:], in_=mask_d[:])
            lnw_sb = consts.tile([128, C], F32)
            nc.sync.dma_start(out=lnw_sb[:], in_=bass.AP(
                tensor=lnw_d.tensor, offset=lnw_d.offset,
                ap=[[0, 128]] + list(lnw_d.ap)))
            lnb_sb = consts.tile([128, C], F32)
            nc.sync.dma_start(out=lnb_sb[:], in_=bass.AP(
                tensor=lnb_d.tensor, offset=lnb_d.offset,
                ap=[[0, 128]] + list(lnb_d.ap)))
            ones_sb = consts.tile([1, 64], F32R)
            nc.sync.dma_start(out=ones_sb[:], in_=ones_d[:])
            eps_sb = consts.tile([128, 1], F32)
            nc.vector.memset(eps_sb[:], EPS)
            boff_sb = consts.tile([1, 1], mybir.dt.uint32)
            nc.sync.dma_start(out=boff_sb[:], in_=boff_d[:])
            warm_in = dram.tile([8, 32], F32, name="warm_in")
            warm_out = dram.tile([8, 32], F32, name="warm_out")
            warm_sb = consts.tile([8, 32], F32, name="warm_sb")
            nc.vector.memset(warm_sb[:], 0.0)
            nc.gpsimd.dma_start(out=warm_in[:], in_=warm_sb[:])
            nc.gpsimd.collective_compute(
                "AllToAll", mybir.AluOpType.bypass,
                replica_groups=[[0, 1, 2, 3, 4, 5, 6, 7]],
                ins=[warm_in.opt()], outs=[warm_out.opt()],
            )

            # ---- persistent activations -----------------------------------
            qrot = persist.tile([128, 2, T], BF16)   # [:, i, :] = heads 2i,2i+1
            krot = persist.tile([128, 2, T], BF16)
            v_sb = persist.tile([128, NTB, HPC, D + 1], BF16)
            nc.vector.memset(v_sb[:, :, :, D:D + 1], 1.0)

            # ---- phase 1 (interleaved): QKV projection + RoPE, and V ------
            # col blocks 0..1: q heads (01),(23); 2..3: k; cols 512:768: v
            def qkv_tile(tt):
                if tt == 0:
                    xt = xt0
                else:
                    xt = work.tile([128, 8, TT], BF16, name="xt", bufs=2)
                    for kc in range(8):
                        nc.sync.dma_start(
                            out=xt[:, kc, :],
                            in_=xT_d.rearrange("(kc p) t -> p kc t", p=128)[
                                :, kc, tt * TT:(tt + 1) * TT],
                        )
                for cb in range(4):
                    qk_ps = ps.tile([128, TT], F32, name="qk_ps", tag="big1024",
                                    bufs=2, padded_shape=[128, 1024])
                    for kc in range(8):
                        nc.tensor.matmul(
                            qk_ps[:],
                            wqkv_sb[:, kc, cb * 128:(cb + 1) * 128],
                            xt[:, kc, :],
                            start=(kc == 0),
                            stop=(kc == 7),
                        )
                    qkT = work.tile([128, TT], F32R, name="qkT", bufs=2)
                    nc.scalar.copy(qkT[:], qk_ps[:])
                    u_ps = ps.tile([128, TT], F32, name="u_ps", tag="reptag",
                                   bufs=2)
                    nc.tensor.matmul(u_ps[:], r_sb[:], qkT[:],
                                     start=True, stop=True)
                    dest = (qrot if cb < 2 else krot)[:, cb % 2,
                                                      tt * TT:(tt + 1) * TT]
                    csl = c_sb[:, tt * TT:(tt + 1) * TT]
                    ssl = s_sb[:, tt * TT:(tt + 1) * TT]
                    tmp = work.tile([128, TT], F32, name="tmp", bufs=2)
                    tmp2 = work.tile([128, TT], F32, name="tmp2", bufs=2)
                    nc.vector.tensor_mul(tmp[:], qkT[:], csl)
                    nc.vector.tensor_mul(tmp2[:], u_ps[:], ssl)
                    nc.vector.tensor_add(dest, tmp[:], tmp2[:])
                # V for the 4 token blocks of this token tile
                for tb in range(tt * 4, tt * 4 + 4):
                    v_ps = ps.tile([128, DC], F32, name="v_ps", tag="big1024",
                                   bufs=2, padded_shape=[128, 1024])
                    for kc in range(8):
                        nc.tensor.matmul(
                            v_ps[:],
                            xt[:, kc, (tb % 4) * 128:(tb % 4 + 1) * 128],
                            wqkv_sb[:, kc, 2 * DC:3 * DC],
                            start=(kc == 0),
                            stop=(kc == 7),
                        )
                    nc.scalar.copy(
                        v_sb[:, tb, :, 0:D],
                        v_ps[:].rearrange("p (h d) -> p h d", h=HPC),
                    )

            qkv_tile(0)

            # weights for c_proj (first needed ~halfway through attention)
            wp_sb = consts.tile([128, 8, C], F32R)
            nc.sync.dma_start(out=wp_sb[:],
                              in_=wpT_d.rearrange("(kc p) n -> p kc n", p=128))

            # ---- phase 2: per 512-token chunk: attention (4 heads) +
            #      c_proj partial + chunked ReduceScatter + LayerNorm -------
            scale = 1.0 / float(np.sqrt(D))
            boff_reg = nc.gpsimd.alloc_register("blkoff")
            nc.gpsimd.reg_load(boff_reg, boff_sb[0:1, 0:1])
            boff_val = nc.gpsimd.snap(boff_reg, donate=True, min_val=0,
                                      max_val=4)
            pend = None

            def cproj_ln(a2a_outs, qt, att_mm, tail=False):
                yTg = work.tile([128, 8, 128], F32R, name="yTg", bufs=2)
                for h2 in range(2):
                    ld = nc.gpsimd.dma_start(
                        out=yTg[:].rearrange("p (r hh) t -> p r hh t",
                                             hh=2)[:, :, h2, :].bitcast(F32),
                        in_=a2a_outs[h2][bass.ds(boff_val, 4)].rearrange(
                            "r p t -> p r t"),
                    )
                    add_dep_helper(ld.ins, att_mm.ins, sync=False,
                                   reason="pipeline c_proj one chunk behind")
                ln = work.tile([128, C], F32, name="ln", bufs=2)
                for oc in range(2):
                    co_ps = ps.tile([128, 512], F32, name="co_ps",
                                    tag="big1024", bufs=2,
                                    padded_shape=[128, 1024])
                    for kc in range(8):
                        nc.tensor.matmul(
                            co_ps[:],
                            yTg[:, kc, :],
                            wp_sb[:, kc, oc * 512:(oc + 1) * 512],
                            start=(kc == 0),
                            stop=(kc == 7),
                        )
                    nc.vector.tensor_copy(ln[:, oc * 512:(oc + 1) * 512],
                                          co_ps[:])
                stats = work.tile([128, 2, 6], F32, name="stats", bufs=2)
                nc.vector.bn_stats(stats[:, 0, :], ln[:, 0:512])
                nc.vector.bn_stats(stats[:, 1, :], ln[:, 512:1024])
                mv = work.tile([128, 2], F32, name="mv", bufs=2)
                nc.vector.bn_aggr(mv[:], stats[:])
                negmean = work.tile([128, 1], F32, name="negmean", bufs=2)
                nc.vector.tensor_scalar_mul(negmean[:], mv[:, 0:1], -1.0)
                std = work.tile([128, 1], F32, name="std", bufs=2)
                nc.scalar.activation(std[:], mv[:, 1:2],
                                     mybir.ActivationFunctionType.Sqrt,
                                     bias=eps_sb[:])
                rstd = work.tile([128, 1], F32, name="rstd", bufs=2)
                nc.vector.reciprocal_approx_fast(rstd[:], std[:])
                nc.vector.tensor_scalar(
                    ln[:], ln[:], negmean[:], rstd[:],
                    op0=mybir.AluOpType.add, op1=mybir.AluOpType.mult,
                )
                eng = nc.vector if tail else nc.gpsimd
                eng.tensor_mul(ln[:], ln[:], lnw_sb[:])
                eng.tensor_add(ln[:], ln[:], lnb_sb[:])
                nc.gpsimd.dma_start(out=out_d[qt * 128:(qt + 1) * 128, :],
                                    in_=ln[:])
            for qt in (0, 1, 2, 3):
                y_qt = work.tile([128, 2, TT], F32R, name="y_qt", bufs=2)
                a2a_outs = []
                for hp in range(2):
                    hA, hB = 2 * hp, 2 * hp + 1
                    qsrc = qrot[:, hp, :]
                    ksrc = krot[:, hp, :]
                    y_psA = ps.tile([D + 1, TT], F32, name="y_psA",
                                    tag="ytag", bufs=2)
                    y_psB = ps.tile([D + 1, TT], F32, name="y_psB",
                                    tag="ytag", bufs=2)
                    for kb in range(4 * qt + 4):
                        s_off = kb - 4 * qt      # >= 0 -> diagonal block
                        q_lo = 128 * s_off if s_off > 0 else 0
                        sc_ps = ps.tile([128, 2, TT], F32, name="sc_ps",
                                        tag="big1024", bufs=2)
                        nc.tensor.matmul(
                            sc_ps[:, 0, q_lo:],
                            ksrc[0:64, kb * 128:(kb + 1) * 128],
                            qsrc[0:64, qt * TT + q_lo:(qt + 1) * TT],
                            start=True, stop=True,
                        )
                        nc.tensor.matmul(
                            sc_ps[:, 1, q_lo:],
                            ksrc[64:128, kb * 128:(kb + 1) * 128],
                            qsrc[64:128, qt * TT + q_lo:(qt + 1) * TT],
                            start=True, stop=True,
                        )
                        ex = work.tile([128, 2, TT], BF16, name="ex", bufs=3)
                        last_exp = nc.scalar.activation(
                            ex[:, :, q_lo:], sc_ps[:, :, q_lo:],
                            mybir.ActivationFunctionType.Exp, scale=scale,
                        )
                        if s_off >= 0:
                            nc.vector.tensor_mul(
                                ex[:, 0, q_lo:q_lo + 128],
                                ex[:, 0, q_lo:q_lo + 128],
                                mask_sb[:],
                            )
                            nc.vector.tensor_mul(
                                ex[:, 1, q_lo:q_lo + 128],
                                ex[:, 1, q_lo:q_lo + 128],
                                mask_sb[:],
                            )
                        nc.tensor.matmul(
                            y_psA[:, q_lo:],
                            v_sb[:, kb, hA, :],
                            ex[:, 0, q_lo:],
                            start=(kb == 0),
                            stop=(kb == 4 * qt + 3),
                            skip_group_check=True,
                        )
                        last_att_mm = nc.tensor.matmul(
                            y_psB[:, q_lo:],
                            v_sb[:, kb, hB, :],
                            ex[:, 1, q_lo:],
                            start=(kb == 0),
                            stop=(kb == 4 * qt + 3),
                            skip_group_check=True,
                        )
                    # normalize each head: y / denom (denom = row D of y_ps)
                    for half, y_ps in ((0, y_psA), (1, y_psB)):
                        den = work.tile([1, TT], F32R, name="den", bufs=2)
                        nc.vector.tensor_copy(den[:], y_ps[D:D + 1, :])
                        rep_ps = ps.tile([64, TT], F32, name="rep_ps",
                                         tag="reptag", bufs=2)
                        nc.tensor.matmul(rep_ps[:], ones_sb[:], den[:],
                                         start=True, stop=True)
                        rrec = work.tile([64, TT], F32, name="rrec", bufs=2)
                        nc.vector.reciprocal_approx_fast(rrec[:], rep_ps[:])
                        nc.vector.tensor_mul(
                            y_qt[64 * half:64 * half + 64, hp, :],
                            y_ps[0:D, :],
                            rrec[:],
                        )
                    # ship this head-pair's y through AllToAll: every core
                    # gets these 128 y-dims for its own 128-token slice
                    a2a_in = dram.tile([8, 128, 128], F32,
                                       name=f"a2a_in{qt}_{hp}")
                    for half in range(2):
                        nc.gpsimd.dma_start(
                            out=a2a_in[4 * half:4 * half + 4].rearrange(
                                "r p t -> p r t"),
                            in_=y_qt[:, hp, :].rearrange(
                                "p (r t) -> p r t", r=4).bitcast(F32),
                        )
                    a2a_out = dram.tile([8, 128, 128], F32,
                                        name=f"a2a_out{qt}_{hp}")
                    a2a_outs.append(a2a_out)
                    nc.gpsimd.collective_compute(
                        "AllToAll",
                        mybir.AluOpType.bypass,
                        replica_groups=[[0, 1, 2, 3, 4, 5, 6, 7]],
                        ins=[a2a_in.opt()],
                        outs=[a2a_out.opt()],
                    )

                if qt < 3:
                    qkv_tile(qt + 1)
                if pend is not None:
                    cproj_ln(*pend, last_att_mm)
                pend = (a2a_outs, qt)
            cproj_ln(*pend, last_att_mm, tail=True)

    nc.compile()
    return nc


_PERM64 = np.concatenate([np.arange(0, 64, 2), np.arange(1, 64, 2)])


def _host_prep(x, rope_freqs, W_attn, W_proj, ln_weight, ln_bias):
    """Build the 8 per-core input maps."""
    import ml_dtypes

    x = np.ascontiguousarray(np.asarray(x, dtype=np.float32))
    W_attn = np.asarray(W_attn, dtype=np.float32)
    W_proj = np.asarray(W_proj, dtype=np.float32)
    rope_freqs = np.asarray(rope_freqs, dtype=np.float32)
    ln_weight = np.ascontiguousarray(np.asarray(ln_weight, dtype=np.float32))
    ln_bias = np.ascontiguousarray(np.asarray(ln_bias, dtype=np.float32))

    # RoPE tables, [128, T]: row r uses pair frequency freqs[r % 32]
    t = np.arange(T, dtype=np.float32)
    theta = t[None, :] * rope_freqs[np.arange(128) % 32][:, None]
    c128 = np.cos(theta).astype(np.float32)
    s128 = np.sin(theta).astype(np.float32)

    # rotation helper matrix (lhsT layout [K, M]), per 64-row head block
    rmat = np.zeros((128, 128), dtype=np.float32)
    for j2 in range(2):
        base = 64 * j2
        for r in range(32):
            rmat[base + 32 + r, base + r] = -1.0
            rmat[base + r, base + 32 + r] = 1.0

    mask = np.triu(np.ones((128, 128))).astype(ml_dtypes.bfloat16)

    xT = [np.ascontiguousarray(x[b].T).astype(ml_dtypes.bfloat16) for b in range(B)]
    wpT_full = np.ascontiguousarray(W_proj.T)

    in_maps = []
    for c in range(N_CORES):
        b, g = c // 4, c % 4
        heads = range(4 * g, 4 * g + 4)
        wq = np.concatenate([W_attn[h * D + _PERM64] for h in heads])
        wk = np.concatenate([W_attn[C + h * D + _PERM64] for h in heads])
        wv = W_attn[2 * C + 4 * g * D:2 * C + (4 * g + 4) * D]
        wqkvT = np.ascontiguousarray(np.concatenate([wq, wk, wv]).T).astype(ml_dtypes.bfloat16)

        in_maps.append({
            "xT": xT[b],
            "wqkvT": wqkvT,
            "wpT": wpT_full,
            "c128": c128,
            "s128": s128,
            "rmat": rmat,
            "mask": mask,
            "ln_w": ln_weight,
            "ln_b": ln_bias,
            "ones": np.ones((1, 64), dtype=np.float32),
            "blk_off": np.array([[4 * b]], dtype=np.uint32),
        })
    return in_maps


_NC_CACHE = None


def kernel(x, rope_freqs, W_attn, W_proj, ln_weight, ln_bias):
    global _NC_CACHE
    if _NC_CACHE is None:
        _NC_CACHE = build()
    nc = _NC_CACHE
    in_maps = _host_prep(x, rope_freqs, W_attn, W_proj, ln_weight, ln_bias)
    res = None
    for attempt in range(3):
        try:
            res = run_bass_kernel_spmd(nc, in_maps,
                                       core_ids=list(range(N_CORES)))
            break
        except Exception:
            if attempt == 2:
                raise
    out = np.empty((B, T, C), dtype=np.float32)
    for c in range(N_CORES):
        b, g = c // 4, c % 4
        chunk = res.results[c]["out"]     # [512, 1024]: 4 chunks of 128 rows
        for qt in range(NTT):
            out[b, qt * 512 + g * 128:qt * 512 + g * 128 + 128, :] = \
                chunk[qt * 128:(qt + 1) * 128]
    return out
